# revision 26
# baseline (speedup 1.0000x reference)
"""Trainium2 Bass kernel for nn_Align_54279796687162 (sparse_attention).

Pure data parallel: one sample per NeuronCore (B=8 over 8 cores).
Per-core layout: activations channel-major [C(partitions), n = h*128 + w].
bf16 matmul inputs, f32 PSUM accumulation.

Self-contained: hardcodes shapes, builds the Bass/Tile graph, shards inputs,
runs via run_bass_kernel_spmd on cores 0-7, gathers the full output.
"""

import numpy as np
import ml_dtypes

import concourse.bass as bass
import concourse.mybir as mybir
import concourse.tile as tile
from concourse import bacc
from concourse.bass_utils import run_bass_kernel_spmd

BF = mybir.dt.bfloat16
F32 = mybir.dt.float32
AF = mybir.ActivationFunctionType
ALU = mybir.AluOpType
AX = mybir.AxisListType

H = W = 128
N = H * W            # 16384
NB = 32              # spatial blocks of 4 rows
BL = 512             # block size (4 rows * 128)
CH = 4               # chunks
SCALE = 0.25         # KD ** -0.5
PST = 132            # padded row stride for q/k/v (DW conv layout)
PSZ = PST * 130      # padded tensor size per partition

# bias column map in the packed [128, 20] f32 bias tile
B_CCAM, B_ENC, B_Q, B_K, B_V, B_DW, B_PW, B_ROW, B_COL, B_PROJ3 = (
    0, 2, 3, 4, 5, 7, 11, 13, 15, 17)

_CACHE = {}


def _ap(base, extra_off, free_dims):
    """Build an AP from a tile's base AP with custom free dims."""
    b = base[:]
    return bass.AP(b.tensor, b.offset + extra_off, [list(b.ap[0])] + free_dims)


def build_graph(scale_ccam: float):
    nc = bacc.Bacc(None, target_bir_lowering=False)

    xb = nc.dram_tensor("xb", [128, N], F32, kind="ExternalInput")
    w3t = nc.dram_tensor("w3t", [128, 9 * 256], BF, kind="ExternalInput")
    wenc = nc.dram_tensor("wenc", [128, 32], BF, kind="ExternalInput")
    wq = nc.dram_tensor("wq", [128, 256], BF, kind="ExternalInput")
    wk = nc.dram_tensor("wk", [128, 256], BF, kind="ExternalInput")
    wv = nc.dram_tensor("wv", [128, 512], BF, kind="ExternalInput")
    wqs = nc.dram_tensor("wqs", [128, 256], BF, kind="ExternalInput")
    wks = nc.dram_tensor("wks", [128, 256], BF, kind="ExternalInput")
    wvs = nc.dram_tensor("wvs", [128, 512], BF, kind="ExternalInput")
    dwd = nc.dram_tensor("dwd", [128, 36 * 128], BF, kind="ExternalInput")
    wpw = nc.dram_tensor("wpw", [128, 4 * 256], BF, kind="ExternalInput")
    wrow = nc.dram_tensor("wrow", [128, 512], BF, kind="ExternalInput")
    wcol = nc.dram_tensor("wcol", [128, 512], BF, kind="ExternalInput")
    wproj = nc.dram_tensor("wproj", [128, 512], BF, kind="ExternalInput")
    post = nc.dram_tensor("post", [16, 4 * 512], BF, kind="ExternalInput")
    interpm = nc.dram_tensor("interpm", [16, 128], BF, kind="ExternalInput")
    identb = nc.dram_tensor("identb", [128, 128], BF, kind="ExternalInput")
    identf = nc.dram_tensor("identf", [128, 128], F32, kind="ExternalInput")
    onesb = nc.dram_tensor("onesb", [128, 1], BF, kind="ExternalInput")
    biases = nc.dram_tensor("biases", [128, 20], F32, kind="ExternalInput")
    dwsc = nc.dram_tensor("dwsc", [128, 36], F32, kind="ExternalInput")

    cb_dram = nc.dram_tensor("cb_dram", [2, 128, N], BF, kind="Internal")
    cf_dram = nc.dram_tensor("cf_dram", [16, N], BF, kind="Internal")
    qo_dram = nc.dram_tensor("qo_dram", [2, 128, N], BF, kind="Internal")
    out = nc.dram_tensor("out", [256, N], F32, kind="ExternalOutput")

    with tile.TileContext(nc) as tc:
      with tc.tile_pool(name="cst", bufs=1) as cst:
        w3_s = cst.tile([128, 9 * 256], BF)
        wenc_s = cst.tile([128, 32], BF)
        wq_s = cst.tile([128, 256], BF)
        wk_s = cst.tile([128, 256], BF)
        wv_s = cst.tile([128, 512], BF)
        wqs_s = cst.tile([128, 256], BF)
        wks_s = cst.tile([128, 256], BF)
        wvs_s = cst.tile([128, 512], BF)
        dwd_s = cst.tile([128, 36 * 128], BF)
        wpw_s = cst.tile([128, 4 * 256], BF)
        wrow_s = cst.tile([128, 512], BF)
        wcol_s = cst.tile([128, 512], BF)
        wproj_s = cst.tile([128, 512], BF)
        post_s = cst.tile([16, 4 * 512], BF)
        interp_s = cst.tile([16, 128], BF)
        idb_s = cst.tile([128, 128], BF)
        idf_s = cst.tile([128, 128], F32)
        ones_s = cst.tile([128, 1], BF)
        bia_s = cst.tile([128, 20], F32)
        dwsc_s = cst.tile([128, 36], F32)
        for t, d in [(w3_s, w3t), (wenc_s, wenc), (wq_s, wq), (wk_s, wk),
                     (wv_s, wv), (wqs_s, wqs), (wks_s, wks), (wvs_s, wvs),
                     (dwd_s, dwd), (wpw_s, wpw), (wrow_s, wrow),
                     (wcol_s, wcol), (wproj_s, wproj), (post_s, post),
                     (interp_s, interpm), (idb_s, identb), (idf_s, identf),
                     (ones_s, onesb), (bia_s, biases), (dwsc_s, dwsc)]:
            nc.sync.dma_start(t[:], d[:])

        attnT_s = cst.tile([16, 256], BF)
        xfs_row = [cst.tile([128, 512], BF, tag=f"xfsr{h}", name=f"xfsr{h}") for h in range(2)]
        xfs_col = [cst.tile([128, 512], F32, tag=f"xfsc{h}", name=f"xfsc{h}") for h in range(2)]
        xproj = {(d_, t_): cst.tile([128, 512], BF, tag=f"xp{d_}{t_}", name=f"xp{d_}{t_}")
                 for d_ in range(2) for t_ in range(2)}

        # =========================================================
        # Phase A: conv3x3 -> ccam_b ; ccam_f ; energy ; ccam attn
        # =========================================================
        with (
            tc.tile_pool(name="pa", bufs=1) as pa,
            tc.tile_pool(name="par", bufs=3) as par,
            tc.tile_pool(name="pamm", bufs=4, space="PSUM") as pamm,
            tc.tile_pool(name="patr", bufs=3, space="PSUM") as patr,
            tc.tile_pool(name="pae", bufs=1, space="PSUM") as pae,
        ):
            xpad = pa.tile([128, 130 * 130], BF)
            cb = [pa.tile([128, N], BF, tag=f"cb{h}", name=f"cb{h}") for h in range(2)]
            cf = pa.tile([16, N], BF)

            nc.gpsimd.memset(xpad[:], 0.0)
            xstage = pa.tile([128, N], BF)
            nc.gpsimd.dma_start(xstage[:], xb[:])
            nc.vector.tensor_copy(
                _ap(xpad, 131, [[130, 128], [1, 128]]), xstage[:])

            # conv3x3: contiguous padded windows (junk cols stripped by
            # the ACT extraction copy), tap-major over 4-block psum groups
            cblk = [(r0, 3) for r0 in range(0, 126, 3)] + [(126, 2)]
            for half in range(2):
                for g0 in range(0, len(cblk), 4):
                    grp = cblk[g0:g0 + 4]
                    pss = [pamm.tile([128, BL], F32, tag="amm",
                                     name=f"cps{j}")
                           for j in range(len(grp))]
                    for t9 in range(9):
                        ky, kx = divmod(t9, 3)
                        for j, (r0, nr) in enumerate(grp):
                            rhs = _ap(xpad, (r0 + ky) * 130 + kx,
                                      [[1, nr * 130 - 2]])
                            nc.tensor.matmul(
                                _ap(pss[j], 0, [[1, nr * 130 - 2]]),
                                w3_s[:, t9 * 256 + half * 128:
                                     t9 * 256 + half * 128 + 128],
                                rhs, start=(t9 == 0), stop=(t9 == 8))
                    for j, (r0, nr) in enumerate(grp):
                        nc.scalar.activation(
                            cb[half][:, r0 * 128:(r0 + nr) * 128],
                            _ap(pss[j], 0, [[130, nr], [1, 128]]),
                            AF.Relu,
                            bias=bia_s[:, B_CCAM + half:B_CCAM + half + 1])
                nc.sync.dma_start(cb_dram[half, :, :], cb[half][:])

            # ccam_f = relu(w_enc @ ccam_b + b_enc)  -> [16, N]
            for bg in range(8):
                pss = [pamm.tile([16, BL], F32, tag="amm",
                                 name=f"fps{j}") for j in range(4)]
                for half in range(2):
                    for j in range(4):
                        b = bg * 4 + j
                        nc.tensor.matmul(
                            pss[j][:], wenc_s[:, half * 16:half * 16 + 16],
                            cb[half][:, b * BL:(b + 1) * BL],
                            start=(half == 0), stop=(half == 1))
                for j in range(4):
                    b = bg * 4 + j
                    nc.scalar.activation(
                        cf[:, b * BL:(b + 1) * BL], pss[j][:], AF.Relu,
                        bias=bia_s[:16, B_ENC:B_ENC + 1])
            nc.sync.dma_start(cf_dram[:, :], cf[:])

            # energy^T [16, 256] accumulated over 128 column-blocks.
            # All three transposes share one PSUM bank (disjoint columns),
            # evacuated by a single ACT copy.
            e_ps = pae.tile([16, 256], F32)
            for b in range(128):
                sl = slice(b * 128, (b + 1) * 128)
                tball = patr.tile([128, 272], BF, tag="tr")
                nc.tensor.matmul(tball[:, 0:128], cb[0][:, sl], idb_s[:],
                                 is_transpose=True, start=True, stop=False)
                nc.tensor.matmul(tball[:, 128:256], cb[1][:, sl], idb_s[:],
                                 is_transpose=True, start=False, stop=False)
                nc.tensor.matmul(tball[:, 256:272], cf[:, sl],
                                 idb_s[:16, :16],
                                 is_transpose=True, start=False, stop=True)
                bT = par.tile([128, 272], BF, tag="bT")
                nc.scalar.activation(bT[:], tball[:], AF.Copy)
                nc.tensor.matmul(e_ps[:], bT[:, 256:272], bT[:, 0:256],
                                 start=(b == 0), stop=(b == 127))

            # CCAM attention: attn = softmax(-energy) over K=16, store attn^T
            e_sb = pa.tile([16, 256], F32)
            nc.scalar.activation(e_sb[:], e_ps[:], AF.Copy)
            for half in range(2):
                tps = patr.tile([128, 16], F32, tag="tr")
                nc.tensor.transpose(
                    tps[:], e_sb[:, half * 128:(half + 1) * 128],
                    idf_s[:16, :16])
                e_c = par.tile([128, 16], F32, tag="ec")
                nc.vector.tensor_copy(e_c[:], tps[:])
                mn = par.tile([128, 1], F32, tag="mn")
                nc.vector.tensor_reduce(mn[:], e_c[:], axis=AX.X, op=ALU.min)
                ex = par.tile([128, 16], F32, tag="ex")
                nc.scalar.activation(ex[:], e_c[:], AF.Exp,
                                     bias=mn[:], scale=-1.0)
                sm = par.tile([128, 1], F32, tag="sm")
                nc.vector.tensor_reduce(sm[:], ex[:], axis=AX.X, op=ALU.add)
                rc = par.tile([128, 1], F32, tag="rc")
                nc.vector.reciprocal(rc[:], sm[:])
                at = par.tile([128, 16], BF, tag="at")
                nc.vector.tensor_scalar(at[:], ex[:], rc[:],
                                        float(scale_ccam), ALU.mult, ALU.mult)
                tat = patr.tile([16, 128], BF, tag="tr")
                nc.tensor.transpose(tat[:], at[:], idb_s[:])
                nc.vector.tensor_copy(
                    attnT_s[:, half * 128:(half + 1) * 128], tat[:])

        # =========================================================
        # Phase B: xf blocks -> shunts + q/k/v ; then DW + PW
        # =========================================================
        with tc.tile_pool(name="pv", bufs=1) as pv:
          v_sb = [pv.tile([128, PSZ], BF, tag=f"v{h}", name=f"v{h}")
                  for h in range(2)]
          with (
              tc.tile_pool(name="pb", bufs=1) as pb,
              tc.tile_pool(name="pbr", bufs=3) as pbr,
              tc.tile_pool(name="pbmm", bufs=4, space="PSUM") as pbmm,
          ):
            q_sb = pb.tile([128, PSZ], BF)
            k_sb = pb.tile([128, PSZ], BF)
            for t_ in [q_sb, k_sb, v_sb[0], v_sb[1]]:
                nc.gpsimd.memset(t_[:], 0.0)

            for bg in range(8):
                xfg = []
                for j in range(4):
                    b = bg * 4 + j
                    sl = slice(b * BL, (b + 1) * BL)
                    cbi = [pbr.tile([128, BL], BF, tag=f"cbi{h}",
                                    name=f"cbi{h}", bufs=2) for h in range(2)]
                    cfi = pbr.tile([16, BL], BF, tag="cfi")
                    nc.gpsimd.dma_start(cbi[0][:], cb_dram[0, :, sl])
                    nc.gpsimd.dma_start(cbi[1][:], cb_dram[1, :, sl])
                    nc.gpsimd.dma_start(cfi[:], cf_dram[:, sl])

                    xf = [pbr.tile([128, BL], BF, tag=f"xf{h}",
                                   name=f"xf{h}", bufs=4) for h in range(2)]
                    for half in range(2):
                        co_ps = pbmm.tile([128, BL], F32, tag="bmm")
                        nc.tensor.matmul(
                            co_ps[:], attnT_s[:, half * 128:(half + 1) * 128],
                            cfi[:], start=True, stop=True)
                        co_sb = pbr.tile([128, BL], BF, tag="cosb",
                                         name="co_sb")
                        nc.scalar.activation(co_sb[:], co_ps[:], AF.Copy)
                        nc.vector.tensor_tensor(
                            xf[half][:], co_sb[:], cbi[half][:], ALU.add)

                    for half in range(2):
                        with nc.allow_low_precision(reason="bf16 shunt sums"):
                            src = _ap(xf[half], 0,
                                      [[1, 4], [128, 4], [4, 32]])
                            dst = _ap(xfs_row[half], 4 * b,
                                      [[128, 4], [1, 4]])
                            nc.vector.tensor_reduce(dst, src, axis=AX.X,
                                                    op=ALU.add)
                        ci = b // 8
                        part = pbr.tile([128, 128], F32, tag=f"cp{half}",
                                        name=f"cp{half}", bufs=2)
                        src = _ap(xf[half], 0, [[1, 128], [128, 4]])
                        nc.vector.tensor_reduce(part[:], src, axis=AX.X,
                                                op=ALU.add)
                        dstc = xfs_col[half][:, ci * 128:(ci + 1) * 128]
                        if b % 8 == 0:
                            nc.gpsimd.tensor_copy(dstc, part[:])
                        else:
                            nc.gpsimd.tensor_tensor(dstc, dstc, part[:],
                                                    ALU.add)
                    xfg.append(xf)

                for (dsts, wt, bc, nt) in [([q_sb], wq_s, B_Q, 1),
                                           ([k_sb], wk_s, B_K, 1),
                                           (v_sb, wv_s, B_V, 2)]:
                    for mt in range(nt):
                        pss = [pbmm.tile([128, BL], F32, tag="bmm",
                                         name=f"qps{j}") for j in range(4)]
                        for kh in range(2):
                            for j in range(4):
                                nc.tensor.matmul(
                                    pss[j][:],
                                    wt[:, (kh * nt + mt) * 128:
                                       (kh * nt + mt) * 128 + 128],
                                    xfg[j][kh][:], start=(kh == 0),
                                    stop=(kh == 1))
                        for j in range(4):
                            b = bg * 4 + j
                            pdst = _ap(dsts[mt], (4 * b + 1) * PST + 2,
                                       [[PST, 4], [1, 128]])
                            nc.scalar.activation(
                                pdst, pss[j][:], AF.Identity,
                                bias=bia_s[:, bc + mt:bc + mt + 1])

            # depthwise 3x3 via diagonal matmuls on the padded layout:
            # every tap is a contiguous full-width matmul; pad columns carry
            # zeros so no clipping is needed. 3-row windows (N=392).
            srcs = [q_sb, k_sb, v_sb[0], v_sb[1]]
            taps = [(1, 1), (0, 1), (2, 1), (1, 0), (1, 2),
                    (0, 0), (0, 2), (2, 0), (2, 2)]
            dblk = [(r0, 3) for r0 in range(0, 126, 3)] + [(126, 2)]
            for g0 in range(0, len(dblk), 2):
                grp = dblk[g0:g0 + 2]
                dwg = []
                for t in range(4):
                    pss = [pbmm.tile([128, BL], F32, tag="bmm",
                                     name=f"dps{j}")
                           for j in range(len(grp))]
                    for ti, (ky, kx) in enumerate(taps):
                        tap9 = ky * 3 + kx
                        wsl = dwd_s[:, (t * 9 + tap9) * 128:
                                    (t * 9 + tap9) * 128 + 128]
                        for j, (r0, nr) in enumerate(grp):
                            nn = nr * PST - 4
                            rhs = _ap(srcs[t], (r0 + ky) * PST + kx + 1,
                                      [[1, nn]])
                            nc.tensor.matmul(
                                _ap(pss[j], 0, [[1, nn]]), wsl, rhs,
                                start=(ti == 0), stop=(ti == 8))
                    dwt = [pbr.tile([128, 384], BF, tag=f"dw{t}{j}",
                                    name=f"dw{t}{j}", bufs=2)
                           for j in range(len(grp))]
                    for j, (r0, nr) in enumerate(grp):
                        nc.scalar.activation(
                            dwt[j][:, 0:nr * 128],
                            _ap(pss[j], 0, [[PST, nr], [1, 128]]),
                            AF.Relu,
                            bias=bia_s[:, B_DW + t:B_DW + t + 1])
                    dwg.append(dwt)
                for mt in range(2):
                    pss = [pbmm.tile([128, 384], F32, tag="pwm",
                                     name=f"pps{j}", bufs=2)
                           for j in range(len(grp))]
                    for kt in range(4):
                        wsl = wpw_s[:, kt * 256 + mt * 128:
                                    kt * 256 + mt * 128 + 128]
                        for j, (r0, nr) in enumerate(grp):
                            nc.tensor.matmul(
                                pss[j][:, 0:nr * 128], wsl,
                                dwg[kt][j][:, 0:nr * 128],
                                start=(kt == 0), stop=(kt == 3))
                    for j, (r0, nr) in enumerate(grp):
                        qo = pbr.tile([128, 384], BF, tag="qo")
                        nc.scalar.activation(
                            qo[:, 0:nr * 128], pss[j][:, 0:nr * 128],
                            AF.Identity,
                            bias=bia_s[:, B_PW + mt:B_PW + mt + 1])
                        nc.sync.dma_start(
                            qo_dram[mt, :, r0 * 128:(r0 + nr) * 128],
                            qo[:, 0:nr * 128])

          # =========================================================
          # Phase C1: axial attention (row: dir 0, col: dir 1)
          # =========================================================
          with (
              tc.tile_pool(name="pc", bufs=1) as pc,
              tc.tile_pool(name="pcr", bufs=3) as pcr,
              tc.tile_pool(name="pcmm", bufs=2, space="PSUM") as pcmm,
              tc.tile_pool(name="pcl", bufs=2, space="PSUM") as pcl,
              tc.tile_pool(name="pcav", bufs=2, space="PSUM") as pcav,
              tc.tile_pool(name="pcasm", bufs=2, space="PSUM") as pcasm,
          ):
            for d_ in range(2):
                if d_ == 0:
                    xfs = xfs_row
                else:
                    xfs = [pc.tile([128, 512], BF, tag=f"xfcb{h}", name=f"xfcb{h}")
                           for h in range(2)]
                    for hh in range(2):
                        nc.vector.tensor_copy(xfs[hh][:], xfs_col[hh][:])

                qs_att = pc.tile([128, 512], BF, tag=f"qsa{d_}")
                ks_att = pc.tile([128, 512], BF, tag=f"ksa{d_}")
                vs_att = [pc.tile([128, 512], BF, tag=f"vsa{d_}{h}", name=f"vsa{d_}{h}")
                          for h in range(2)]
                for (dst, wt, bc, nt, pidx) in [
                        ([qs_att], wqs_s, B_Q, 1, 2 * d_),
                        ([ks_att], wks_s, B_K, 1, 2 * d_ + 1),
                        (vs_att, wvs_s, B_V, 2, None)]:
                    for mt in range(nt):
                        ps = pcmm.tile([128, BL], F32, tag="cmm")
                        for kh in range(2):
                            nc.tensor.matmul(
                                ps[:],
                                wt[:, (kh * nt + mt) * 128:
                                   (kh * nt + mt) * 128 + 128],
                                xfs[kh][:], start=(kh == 0),
                                stop=(kh == 1 and pidx is None))
                        if pidx is not None:
                            for i in range(CH):
                                nc.tensor.matmul(
                                    ps[:, i * 128:(i + 1) * 128],
                                    post_s[:, (pidx * 4 + i) * 128:
                                           (pidx * 4 + i) * 128 + 128],
                                    interp_s[:], start=False, stop=(i == 3))
                        nc.scalar.activation(
                            dst[mt][:], ps[:], AF.Identity,
                            bias=bia_s[:, bc + mt:bc + mt + 1])

                # repack q/k to [16(kd), g*512 + i*128 + pos] layout
                q_pack = pc.tile([16, 4096], BF, tag=f"qp{d_}")
                k_pack = pc.tile([16, 4096], BF, tag=f"kp{d_}")
                for g in range(8):
                    nc.sync.dma_start(
                        q_pack[0:16, g * 512:(g + 1) * 512],
                        qs_att[g * 16:(g + 1) * 16, :])
                    nc.sync.dma_start(
                        k_pack[0:16, g * 512:(g + 1) * 512],
                        ks_att[g * 16:(g + 1) * 16, :])

                # v^T per chunk: [128(pos), i, 256(ch2)]
                vt_s = pc.tile([128, 4, 256], BF, tag=f"vt{d_}")
                for i in range(CH):
                    for hh in range(2):
                        tp = pcl.tile([128, 128], BF, tag="lps")
                        nc.tensor.transpose(
                            tp[:], vs_att[hh][:, i * 128:(i + 1) * 128],
                            idb_s[:])
                        nc.scalar.activation(
                            vt_s[:, i, hh * 128:(hh + 1) * 128], tp[:],
                            AF.Copy)

                xpre = [pc.tile([128, 512], BF, tag=f"xpre{d_}{t}", name=f"xpre{d_}{t}")
                        for t in range(2)]
                for i in range(CH):
                    for th in range(2):
                        asm_ps = pcasm.tile([128, 128], BF, tag="asm")
                        for gg in range(4):
                            g = th * 4 + gg
                            sl_gi = slice(g * 512 + i * 128,
                                          g * 512 + i * 128 + 128)
                            l_ps = pcl.tile([128, 128], F32, tag="lps")
                            nc.tensor.matmul(l_ps[:], k_pack[0:16, sl_gi],
                                             q_pack[0:16, sl_gi],
                                             start=True, stop=True)
                            e_t = pcr.tile([128, 128], BF, tag="et")
                            nc.scalar.activation(e_t[:], l_ps[:], AF.Exp,
                                                 scale=SCALE)
                            av_ps = pcav.tile([128, 33], F32, tag="av")
                            nc.tensor.matmul(
                                av_ps[:, 0:32], e_t[:],
                                vt_s[:, i, g * 32:(g + 1) * 32],
                                start=True, stop=False)
                            nc.tensor.matmul(av_ps[:, 32:33], e_t[:],
                                             ones_s[:], start=False, stop=True)
                            rcp = pcr.tile([128, 1], F32, tag="rcp")
                            nc.vector.reciprocal(rcp[:], av_ps[:, 32:33])
                            xrn = pcr.tile([128, 32], BF, tag="xrn")
                            nc.vector.tensor_scalar(
                                xrn[:], av_ps[:, 0:32], rcp[:], None, ALU.mult)
                            nc.tensor.transpose(
                                asm_ps[gg * 32:(gg + 1) * 32, :], xrn[:],
                                idb_s[:], tile_position=(0, gg * 32))
                        nc.scalar.activation(
                            xpre[th][:, i * 128:(i + 1) * 128], asm_ps[:],
                            AF.Relu)

                wproj_d = wrow_s if d_ == 0 else wcol_s
                bcol = B_ROW if d_ == 0 else B_COL
                for mt in range(2):
                    ps = pcmm.tile([128, BL], F32, tag="cmm")
                    for kh in range(2):
                        nc.tensor.matmul(
                            ps[:],
                            wproj_d[:, (kh * 2 + mt) * 128:
                                    (kh * 2 + mt) * 128 + 128],
                            xpre[kh][:], start=(kh == 0), stop=(kh == 1))
                    nc.scalar.activation(
                        xproj[(d_, mt)][:], ps[:], AF.Identity,
                        bias=bia_s[:, bcol + mt:bcol + mt + 1])

          # =========================================================
          # Phase C2: xx = relu(v + bcast(xrow) + bcast(xcol));
          #           att = hsig(proj(xx)+b+3); out = att * qkv
          # =========================================================
          with (
              tc.tile_pool(name="pd", bufs=3) as pd,
              tc.tile_pool(name="pdmm", bufs=5, space="PSUM") as pdmm,
          ):
            for bg in range(8):
                xxg = []
                for j in range(4):
                    b = bg * 4 + j
                    sl = slice(b * BL, (b + 1) * BL)
                    xxr = []
                    for half in range(2):
                        xx = pd.tile([128, BL], BF, tag=f"xx{half}",
                                     name=f"xx{half}", bufs=6)
                        rap = _ap(xproj[(0, half)], b * 16, [[1, 16], [0, 32]])
                        cap = _ap(xproj[(1, half)], (b // 2) * 32,
                                  [[0, 4], [0, 4], [1, 32]])
                        nc.gpsimd.tensor_tensor(xx[:], rap, cap, ALU.add)
                        vap = _ap(v_sb[half], (4 * b + 1) * PST + 2,
                                  [[PST, 4], [1, 128]])
                        nc.vector.tensor_tensor(xx[:], xx[:], vap, ALU.add)
                        nc.vector.tensor_scalar(xx[:], xx[:], 0.0, None,
                                                ALU.max)
                        xxr.append(xx)
                    xxg.append(xxr)
                for mt in range(2):
                    pss = [pdmm.tile([128, BL], F32, tag="dmm",
                                     name=f"jps{j}") for j in range(4)]
                    for kh in range(2):
                        wsl = wproj_s[:, (kh * 2 + mt) * 128:
                                      (kh * 2 + mt) * 128 + 128]
                        for j in range(4):
                            nc.tensor.matmul(
                                pss[j][:], wsl, xxg[j][kh][:],
                                start=(kh == 0), stop=(kh == 1))
                    for j in range(4):
                        b = bg * 4 + j
                        sl = slice(b * BL, (b + 1) * BL)
                        hs = pd.tile([128, BL], F32, tag="hs")
                        nc.scalar.activation(
                            hs[:], pss[j][:], AF.Relu,
                            bias=bia_s[:, B_PROJ3 + mt:B_PROJ3 + mt + 1])
                        att_t = pd.tile([128, BL], BF, tag="att")
                        nc.vector.tensor_scalar(att_t[:], hs[:], 6.0,
                                                1.0 / 6.0, ALU.min, ALU.mult)
                        qo_in = pd.tile([128, BL], BF, tag="qoin")
                        nc.gpsimd.dma_start(qo_in[:], qo_dram[mt, :, sl])
                        ob = pd.tile([128, BL], BF, tag="ob")
                        nc.vector.tensor_tensor(ob[:], att_t[:], qo_in[:],
                                                ALU.mult)
                        nc.gpsimd.dma_start(out[mt * 128:(mt + 1) * 128, sl],
                                            ob[:])

    nc.compile()
    return nc


def _interp_matrix():
    s, n = 16, 128
    src = np.clip((np.arange(n) + 0.5) * (s / n) - 0.5, 0.0, s - 1.0)
    i0 = np.floor(src).astype(np.int64)
    i1 = np.minimum(i0 + 1, s - 1)
    w = src - i0
    M = np.zeros((s, n), np.float64)
    np.add.at(M, (i0, np.arange(n)), 1.0 - w)
    np.add.at(M, (i1, np.arange(n)), w)
    return M


def _bf(x):
    return np.ascontiguousarray(np.asarray(x, np.float32).astype(
        ml_dtypes.bfloat16))


def prep_consts(inputs):
    """Host-side layout prep of all weight tensors (shared across cores)."""
    f = {k: np.asarray(v, np.float32) for k, v in inputs.items()}

    w3 = f["w_ccam_b"]                      # [256, 128, 3, 3]
    w3t = np.zeros((128, 9 * 256), np.float32)
    for ky in range(3):
        for kx in range(3):
            t9 = ky * 3 + kx
            w3t[:, t9 * 256:(t9 + 1) * 256] = w3[:, :, ky, kx].T
    wenc = np.zeros((128, 32), np.float32)  # w_enc [16, 256]
    for half in range(2):
        wenc[:, half * 16:(half + 1) * 16] = \
            f["w_enc"][:, half * 128:(half + 1) * 128].T

    def pack_lhsT(wm, nt):
        # wm [out, in]; returns [128, 2*nt*128]: [ci, (kh*nt+mt)*128+co]
        o, cin = wm.shape
        r = np.zeros((128, 2 * nt * 128), np.float32)
        for kh in range(2):
            for mt in range(nt):
                r[:, (kh * nt + mt) * 128:(kh * nt + mt) * 128 + 128] = \
                    wm[mt * 128:(mt + 1) * 128,
                       kh * 128:(kh + 1) * 128].T
        return r

    wq_p = pack_lhsT(f["w_q"], 1)
    wk_p = pack_lhsT(f["w_k"], 1)
    wv_p = pack_lhsT(f["w_v"], 2)
    wrow_p = pack_lhsT(f["w_row"], 2)
    wcol_p = pack_lhsT(f["w_col"], 2)
    wproj_p = pack_lhsT(f["w_proj"], 2)

    wpw_p = np.zeros((128, 4 * 256), np.float32)   # w_pw [256, 512]
    for kt in range(4):
        for mt in range(2):
            wpw_p[:, kt * 256 + mt * 128:kt * 256 + mt * 128 + 128] = \
                f["w_pw"][mt * 128:(mt + 1) * 128,
                          kt * 128:(kt + 1) * 128].T

    dwdg = np.zeros((128, 36 * 128), np.float32)   # w_dw [512,1,3,3]
    ii = np.arange(128)
    for t in range(4):
        for tap9 in range(9):
            ky, kx = divmod(tap9, 3)
            dwdg[ii, (t * 9 + tap9) * 128 + ii] = \
                f["w_dw"][t * 128 + ii, 0, ky, kx]

    post_p = np.zeros((16, 4 * 512), np.float32)
    for pidx, nm in enumerate(["pos_rowq", "pos_rowk", "pos_colq", "pos_colk"]):
        p = f[nm]                                   # [4, 128, 16]
        for i in range(4):
            post_p[:, (pidx * 4 + i) * 128:(pidx * 4 + i) * 128 + 128] = \
                p[i].T                              # [16, 128]

    biases = np.zeros((128, 20), np.float32)
    biases[:, B_CCAM + 0] = f["b_ccam_b"][:128]
    biases[:, B_CCAM + 1] = f["b_ccam_b"][128:]
    biases[:16, B_ENC] = f["b_enc"]
    biases[:, B_Q] = f["b_q"]
    biases[:, B_K] = f["b_k"]
    biases[:, B_V + 0] = f["b_v"][:128]
    biases[:, B_V + 1] = f["b_v"][128:]
    for t in range(4):
        biases[:, B_DW + t] = f["b_dw"][t * 128:(t + 1) * 128]
    biases[:, B_PW + 0] = f["b_pw"][:128]
    biases[:, B_PW + 1] = f["b_pw"][128:]
    biases[:, B_ROW + 0] = f["b_row"][:128]
    biases[:, B_ROW + 1] = f["b_row"][128:]
    biases[:, B_COL + 0] = f["b_col"][:128]
    biases[:, B_COL + 1] = f["b_col"][128:]
    biases[:, B_PROJ3 + 0] = f["b_proj"][:128] + 3.0
    biases[:, B_PROJ3 + 1] = f["b_proj"][128:] + 3.0

    dwsc_p = np.zeros((128, 36), np.float32)
    for t in range(4):
        for tap9 in range(9):
            ky, kx = divmod(tap9, 3)
            dwsc_p[:, t * 9 + tap9] = f["w_dw"][t * 128:(t + 1) * 128,
                                                0, ky, kx]
    return {
        "dwsc": np.ascontiguousarray(dwsc_p),
        "w3t": _bf(w3t), "wenc": _bf(wenc),
        "wq": _bf(wq_p), "wk": _bf(wk_p), "wv": _bf(wv_p),
        "wqs": _bf(wq_p / 32.0), "wks": _bf(wk_p / 32.0),
        "wvs": _bf(wv_p / 32.0),
        "dwd": _bf(dwdg), "wpw": _bf(wpw_p),
        "wrow": _bf(wrow_p), "wcol": _bf(wcol_p), "wproj": _bf(wproj_p),
        "post": _bf(post_p), "interpm": _bf(_interp_matrix()),
        "identb": _bf(np.eye(128)),
        "identf": np.eye(128, dtype=np.float32),
        "onesb": _bf(np.ones((128, 1))),
        "biases": np.ascontiguousarray(biases),
    }


def kernel(**inputs) -> np.ndarray:
    x = np.asarray(inputs["x"], np.float32)          # [8, 128, 128, 128]
    scale = float(np.asarray(inputs["scale_ccam"]).reshape(-1)[0])

    key = round(scale, 9)
    if key not in _CACHE:
        _CACHE[key] = build_graph(scale)
    nc = _CACHE[key]

    consts = prep_consts(inputs)
    in_maps = []
    for core in range(8):
        m = dict(consts)
        m["xb"] = np.ascontiguousarray(x[core].reshape(128, N))
        in_maps.append(m)

    res = run_bass_kernel_spmd(nc, in_maps, core_ids=list(range(8)))
    outs = [res.results[i]["out"].reshape(256, 128, 128) for i in range(8)]
    return np.stack(outs).astype(np.float32)


if __name__ == "__main__":
    rng = np.random.default_rng(0)
    demo = {"x": rng.standard_normal((8, 128, 128, 128), dtype=np.float32)}
    print("kernel module OK")
